# Initial kernel scaffold
#
"""Trainium2 Bass kernel for nn_CorrelationModule.

Input: x [64, 256, 56, 56] fp32. Output: [64, 3, 56, 56] fp32
(2 motion channels via soft-argmax over a 9x9 correlation cost volume
of L2-normalized features against the previous frame, + max-correlation
confidence channel).

Sharding: pure data parallel over clips (bt = 8 clips x 8 segments);
core k processes clip k.

Per-core algorithm (per frame):
  - load raw frame into a zero-padded 64x64 plane (channel-major)
  - norm^2 per pixel via Square (ACT) + ones-matmul partition reduce (PE)
  - invn = 1/sqrt, broadcast across channel partitions, normalize -> x_m
  - 28 patch-Gram fp32 matmuls: [112 patch pixels] x [352 neighborhood]
    against previous frame's normalized plane
  - masked max (window mask) -> conf; argmax via is_ge * index-map sum
  - decode displacement, write patch-layout output; host re-layouts
"""
import sys
import numpy as np

sys.path.insert(0, '/opt/trn_rl_repo')

import concourse.bass as bass
import concourse.bacc as bacc
import concourse.mybir as mybir
import concourse.tile as tile
from concourse.bass_utils import run_bass_kernel_spmd

F32 = mybir.dt.float32
AF = mybir.ActivationFunctionType
OP = mybir.AluOpType

T, C, H, W = 8, 256, 56, 56
N_CORES = 8
PH, PW = 64, 64          # padded plane dims
PA, PB = 8, 14           # patch rows x cols
NPI, NPJ = H // PA, W // PB
NPATCH = NPI * NPJ       # 28
M = PA * PB              # 112
NR, NC_ = PA + 8, PB + 8 # neighborhood 16 x 22
N = NR * NC_             # 352
NEG = -1.0e30
RCH = 9


def _build(tc, out_ap, in_ap):
    nc = tc.nc
    with tc.tile_pool(name="persist", bufs=1) as pp, \
         tc.tile_pool(name="work", bufs=2) as wp, \
         tc.tile_pool(name="psum", bufs=4, space="PSUM") as ps, \
         tc.tile_pool(name="psn", bufs=2, space="PSUM") as psn:

        raw = [pp.tile([128, PH * PW], F32, tag=f"raw{k}", name=f"raw{k}") for k in range(2)]
        xm = [[pp.tile([128, PH * PW], F32, tag=f"xm{k}_{b}", name=f"xm{k}_{b}")
               for b in range(2)] for k in range(2)]
        xmP = [pp.tile([128, H * W], F32, tag=f"xmP{k}", name=f"xmP{k}") for k in range(2)]
        invn_bc = pp.tile([128, H * W], F32)
        invn_lin = pp.tile([56, 56], F32)
        nrow = pp.tile([1, H * W], F32)
        ones = pp.tile([128, 1], F32)
        mneg = pp.tile([M, N], F32)
        packm = pp.tile([M, N], F32)
        mstore = pp.tile([M, NPATCH], F32)
        dstore = pp.tile([M, NPATCH], F32)
        outbuf = pp.tile([M, 3, NPATCH], F32)

        nc.gpsimd.memset(ones[:], 1.0)
        for k in range(2):
            nc.gpsimd.memset(raw[k][:], 0.0)
            for b in range(2):
                nc.gpsimd.memset(xm[k][b][:], 0.0)

        with tc.tile_pool(name="setup", bufs=1) as sp:
            it_p = sp.tile([M, N], mybir.dt.int32)
            it_nr = sp.tile([M, N], mybir.dt.int32)
            it_nc = sp.tile([M, N], mybir.dt.int32)
            nc.gpsimd.iota(it_p[:], pattern=[[0, N]], base=0, channel_multiplier=1)
            nc.gpsimd.iota(it_nr[:].rearrange("p (a b) -> p a b", a=NR),
                           pattern=[[1, NR], [0, NC_]], base=0, channel_multiplier=0)
            nc.gpsimd.iota(it_nc[:].rearrange("p (a b) -> p a b", a=NR),
                           pattern=[[0, NR], [1, NC_]], base=0, channel_multiplier=0)
            fp_ = sp.tile([M, N], F32)
            fnr = sp.tile([M, N], F32)
            fnc = sp.tile([M, N], F32)
            nc.vector.tensor_copy(fp_[:], it_p[:])
            nc.vector.tensor_copy(fnr[:], it_nr[:])
            nc.vector.tensor_copy(fnc[:], it_nc[:])
            fpj = sp.tile([M, N], F32)
            fpi = sp.tile([M, N], F32)
            nc.gpsimd.memset(fpi[:], 0.0)
            for kk in range(1, PA):
                nc.vector.scalar_tensor_tensor(out=fpi[:], in0=fp_[:], scalar=float(PB * kk),
                                               in1=fpi[:], op0=OP.is_ge, op1=OP.add)
            nc.vector.scalar_tensor_tensor(out=fpj[:], in0=fpi[:], scalar=-float(PB),
                                           in1=fp_[:], op0=OP.mult, op1=OP.add)
            fdi = sp.tile([M, N], F32)
            fdj = sp.tile([M, N], F32)
            nc.vector.tensor_tensor(out=fdi[:], in0=fnr[:], in1=fpi[:], op=OP.subtract)
            nc.vector.tensor_tensor(out=fdj[:], in0=fnc[:], in1=fpj[:], op=OP.subtract)
            msk = sp.tile([M, N], F32)
            tmp = sp.tile([M, N], F32)
            nc.vector.tensor_scalar(out=msk[:], in0=fdi[:], scalar1=0.0, scalar2=None, op0=OP.is_ge)
            nc.vector.tensor_scalar(out=tmp[:], in0=fdi[:], scalar1=8.0, scalar2=None, op0=OP.is_le)
            nc.vector.tensor_tensor(out=msk[:], in0=msk[:], in1=tmp[:], op=OP.mult)
            nc.vector.tensor_scalar(out=tmp[:], in0=fdj[:], scalar1=0.0, scalar2=None, op0=OP.is_ge)
            nc.vector.tensor_tensor(out=msk[:], in0=msk[:], in1=tmp[:], op=OP.mult)
            nc.vector.tensor_scalar(out=tmp[:], in0=fdj[:], scalar1=8.0, scalar2=None, op0=OP.is_le)
            nc.vector.tensor_tensor(out=msk[:], in0=msk[:], in1=tmp[:], op=OP.mult)
            nc.vector.tensor_scalar(out=mneg[:], in0=msk[:], scalar1=1.0, scalar2=-NEG,
                                    op0=OP.subtract, op1=OP.mult)
            nc.vector.tensor_scalar(out=packm[:], in0=fdi[:], scalar1=9.0, scalar2=None, op0=OP.mult)
            nc.vector.tensor_tensor(out=packm[:], in0=packm[:], in1=fdj[:], op=OP.add)
            nc.vector.tensor_tensor(out=packm[:], in0=packm[:], in1=msk[:], op=OP.mult)

        for t in range(T):
            cur, prv = t % 2, 1 - (t % 2)
            for k in range(2):
                nc.sync.dma_start(
                    raw[k][:].rearrange("p (r c) -> p r c", r=PH)[:, 4:4 + H, 4:4 + W],
                    in_ap[t, k * 128:(k + 1) * 128].rearrange("c (r w) -> c r w", r=H),
                )
            rawv = [raw[k][:].rearrange("p (r c) -> p r c", r=PH)[:, 4:4 + H, 4:4 + W]
                    for k in range(2)]
            for r0 in range(0, H, RCH):
                r1 = min(r0 + RCH, H)
                ln = (r1 - r0) * W
                sqc = wp.tile([128, 2, RCH * W], F32, tag="sqc")
                for k in range(2):
                    nc.scalar.activation(
                        sqc[:, k, :ln].rearrange("p (r c) -> p r c", c=W),
                        rawv[k][:, r0:r1], AF.Square)
                nsq = psn.tile([1, RCH * W], F32, tag="nsq")
                for k in range(2):
                    nc.tensor.matmul(nsq[:, :ln], ones[:], sqc[:, k, :ln],
                                     start=(k == 0), stop=(k == 1))
                nc.scalar.activation(nrow[:, r0 * W:r1 * W], nsq[:, :ln], AF.Sqrt)
            nc.sync.dma_start(invn_lin[:], nrow[:].rearrange("o (p j) -> o p j", j=56))
            nc.vector.reciprocal(invn_lin[:], invn_lin[:])
            nc.sync.dma_start(nrow[:].rearrange("o (p j) -> o p j", j=56), invn_lin[:])
            nc.gpsimd.partition_broadcast(invn_bc[:], nrow[:], channels=128)
            for k in range(2):
                xmv = xm[k][cur][:].rearrange("p (r c) -> p r c", r=PH)[:, 4:4 + H, 4:4 + W]
                nc.vector.tensor_tensor(
                    out=xmv, in0=rawv[k],
                    in1=invn_bc[:].rearrange("p (r c) -> p r c", c=W), op=OP.mult)
            for k in range(2):
                src_ap = xm[k][cur][:].rearrange("p (r c) -> p r c", r=PH)[:, 4:4 + H, 4:4 + W] \
                    .rearrange("c (Pi pi) (Pj pj) -> c Pi Pj pi pj", pi=PA, pj=PB)
                nc.gpsimd.tensor_copy(
                    xmP[k][:].rearrange("c (Pi Pj p q) -> c Pi Pj p q", Pi=NPI, Pj=NPJ, p=PA),
                    src_ap)
            if t == 0:
                nc.gpsimd.memset(outbuf[:], 0.0)
                nc.sync.dma_start(out_ap[t], outbuf[:])
                continue
            for P in range(NPATCH):
                Pi, Pj = P // NPJ, P % NPJ
                g = ps.tile([M, N], F32, tag="gram")
                for k in range(2):
                    lhsT = xmP[k][:, P * M:(P + 1) * M]
                    rhs = xm[k][prv][:].rearrange("c (r w) -> c r w", r=PH)[
                        :, Pi * PA: Pi * PA + NR, Pj * PB: Pj * PB + NC_]
                    nc.tensor.matmul(g[:], lhsT, rhs, start=(k == 0), stop=(k == 1))
                gcp = wp.tile([M, N], F32, tag="gcp", bufs=3)
                nc.scalar.activation(gcp[:], g[:], AF.Copy)
                masked = wp.tile([M, N], F32, tag="masked", bufs=3)
                nc.gpsimd.tensor_tensor(out=masked[:], in0=gcp[:], in1=mneg[:], op=OP.add)
                nc.vector.reduce_max(mstore[:, P:P + 1], masked[:], axis=mybir.AxisListType.X)
                junk = wp.tile([M, N], F32, tag="junk")
                nc.vector.scalar_tensor_tensor(
                    out=junk[:], in0=masked[:], scalar=mstore[:, P:P + 1], in1=packm[:],
                    op0=OP.is_ge, op1=OP.mult,
                    accum_out=dstore[:, P:P + 1])
            scl = wp.tile([M, NPATCH], F32, tag="scl")
            dx4 = wp.tile([M, NPATCH], F32, tag="dx4")
            dy4 = wp.tile([M, NPATCH], F32, tag="dy4")
            nc.vector.tensor_scalar(out=scl[:], in0=mstore[:], scalar1=0.0, scalar2=None, op0=OP.is_gt)
            nc.vector.tensor_scalar(out=dy4[:], in0=dstore[:], scalar1=9.0, scalar2=None, op0=OP.is_ge)
            for kk in range(2, 9):
                nc.vector.scalar_tensor_tensor(out=dy4[:], in0=dstore[:], scalar=float(9 * kk),
                                               in1=dy4[:], op0=OP.is_ge, op1=OP.add)
            nc.vector.scalar_tensor_tensor(out=dx4[:], in0=dy4[:], scalar=-9.0,
                                           in1=dstore[:], op0=OP.mult, op1=OP.add)
            nc.vector.tensor_scalar(out=dy4[:], in0=dy4[:], scalar1=-4.0, scalar2=None, op0=OP.add)
            nc.vector.tensor_scalar(out=dx4[:], in0=dx4[:], scalar1=-4.0, scalar2=None, op0=OP.add)
            nc.vector.tensor_tensor(out=outbuf[:, 0, :], in0=dy4[:], in1=scl[:], op=OP.mult)
            nc.vector.tensor_tensor(out=outbuf[:, 1, :], in0=dx4[:], in1=scl[:], op=OP.mult)
            nc.vector.tensor_copy(outbuf[:, 2, :], mstore[:])
            nc.sync.dma_start(out_ap[t], outbuf[:])


_CACHE = {}


def _get_nc():
    if 'nc' in _CACHE:
        return _CACHE['nc']
    from concourse._compat import axon_active
    nc = bacc.Bacc("TRN2", target_bir_lowering=False, debug=not axon_active(),
                   enable_asserts=False)
    x_d = nc.dram_tensor("x_clip", [T, C, H * W], F32, kind="ExternalInput")
    o_d = nc.dram_tensor("out", [T, M, 3, NPATCH], F32, kind="ExternalOutput")
    with tile.TileContext(nc) as tc:
        _build(tc, o_d.ap(), x_d.ap())
    nc.compile()
    _CACHE['nc'] = nc
    return nc


def _patch_to_image(a):
    """[..., 112, 28] patch layout -> [..., 56, 56]"""
    s = a.shape[:-2]
    a = a.reshape(*s, PA, PB, NPI, NPJ)
    a = np.moveaxis(a, (-4, -3, -2, -1), (-3, -1, -4, -2))
    return np.ascontiguousarray(a.reshape(*s, H, W))


def kernel(x, _trace=False, _trace_kwargs=None):
    x = np.ascontiguousarray(x, dtype=np.float32)
    assert x.shape == (64, 256, 56, 56)
    nc = _get_nc()
    xr = x.reshape(N_CORES, T, C, H * W)
    in_maps = [{"x_clip": np.ascontiguousarray(xr[k])} for k in range(N_CORES)]
    res = run_bass_kernel_spmd(nc, in_maps, core_ids=list(range(N_CORES)),
                               trace=_trace, **(_trace_kwargs or {}))
    outs = []
    for k in range(N_CORES):
        o = res.results[k]["out"]                    # [8, 112, 3, 28]
        o = o.transpose(0, 2, 1, 3)                  # [8, 3, 112, 28]
        outs.append(_patch_to_image(o))              # [8, 3, 56, 56]
    full = np.concatenate(outs, axis=0)              # [64, 3, 56, 56]
    if _trace:
        return full, res
    return full



# revision 23
# speedup vs baseline: 1.4082x; 1.4082x over previous
"""Trainium2 Bass kernel for nn_CorrelationModule.

Input: x [64, 256, 56, 56] fp32. Output: [64, 3, 56, 56] fp32
(2 motion channels via soft-argmax over a 9x9 correlation cost volume
of L2-normalized features against the previous frame, + max-correlation
confidence channel).

Sharding: pure data parallel over clips (bt = 8 clips x 8 segments);
core k processes clip k.

v2 changes vs baseline:
  - stationary operand read directly from the normalized plane via a
    strided AP (kills the Pool-engine patchify copy)
  - PSUM-escape + mask-add + window max fused into one DVE
    tensor_tensor_reduce per patch (kills ACT copy + Pool add + DVE max)
  - argmax-extract stt split between DVE and Pool to balance engines
"""
import sys
import numpy as np

sys.path.insert(0, '/opt/trn_rl_repo')

import concourse.bass as bass
import concourse.bacc as bacc
import concourse.mybir as mybir
import concourse.tile as tile
from concourse.bass_utils import run_bass_kernel_spmd

F32 = mybir.dt.float32
F32R = mybir.dt.float32r
AF = mybir.ActivationFunctionType
OP = mybir.AluOpType

T, C, H, W = 8, 256, 56, 56
N_CORES = 8
PH, PW = 64, 64          # padded plane dims
PA, PB = 8, 14           # patch rows x cols
NPI, NPJ = H // PA, W // PB
NPATCH = NPI * NPJ       # 28
M = PA * PB              # 112
NR, NC_ = PA + 8, PB + 8 # neighborhood 16 x 22
N = NR * NC_             # 352
NEG = -1.0e30
CCH = 448                # norm-reduce column chunk (448*7 = 3136)
NCHUNK = (H * W) // CCH
# patches whose argmax-extract stt runs on the Pool engine (rest on DVE)
FUSE_DVE_BATCHES = 6   # first k 4-patch batches use fused DVE mask-add


def _build(tc, out_ap, in_ap):
    nc = tc.nc
    with tc.tile_pool(name="persist", bufs=1) as pp, \
         tc.tile_pool(name="work", bufs=2) as wp, \
         tc.tile_pool(name="psum", bufs=6, space="PSUM") as ps, \
         tc.tile_pool(name="psn", bufs=2, space="PSUM") as psn:

        raw = [pp.tile([128, H * W], F32, tag=f"raw{k}", name=f"raw{k}") for k in range(2)]
        xm = [[pp.tile([128, PH * PW], F32, tag=f"xm{k}_{b}", name=f"xm{k}_{b}")
               for b in range(2)] for k in range(2)]
        xmP = [pp.tile([128, NPJ, H, PB], F32, tag=f"xmP{k}", name=f"xmP{k}")
               for k in range(2)]
        nrow = pp.tile([1, H * W], F32)
        sq_t = [pp.tile([128, 2, CCH], F32, tag=f"sq{cc}", name=f"sq{cc}")
                for cc in range(NCHUNK)]
        invbc = pp.tile([128, H * W], F32)  # row 0 doubles as the invn staging row
        n56 = pp.tile([56, 56], F32)
        ones = pp.tile([128, 1], F32)
        mneg = pp.tile([M, N], F32)
        mneg4 = pp.tile([M, 4, N], F32)
        packm = pp.tile([M, N], F32)
        mstore = [pp.tile([M, NPATCH], F32, tag=f"mst{b}", name=f"mst{b}") for b in range(2)]
        dstore = [pp.tile([M, NPATCH], F32, tag=f"dst{b}", name=f"dst{b}") for b in range(2)]
        outbuf = [pp.tile([M, 3, NPATCH], F32, tag=f"ob{b}", name=f"ob{b}") for b in range(2)]

        nc.gpsimd.memset(ones[:], 1.0)
        for k in range(2):
            for b in range(2):
                nc.gpsimd.memset(xm[k][b][:], 0.0)

        with tc.tile_pool(name="setup", bufs=1) as sp:
            it_p = sp.tile([M, N], mybir.dt.int32)
            it_nr = sp.tile([M, N], mybir.dt.int32)
            it_nc = sp.tile([M, N], mybir.dt.int32)
            nc.gpsimd.iota(it_p[:], pattern=[[0, N]], base=0, channel_multiplier=1)
            nc.gpsimd.iota(it_nr[:].rearrange("p (a b) -> p a b", a=NR),
                           pattern=[[1, NR], [0, NC_]], base=0, channel_multiplier=0)
            nc.gpsimd.iota(it_nc[:].rearrange("p (a b) -> p a b", a=NR),
                           pattern=[[0, NR], [1, NC_]], base=0, channel_multiplier=0)
            fp_ = sp.tile([M, N], F32)
            fnr = it_nr[:].bitcast(F32)
            fnc = it_nc[:].bitcast(F32)
            nc.vector.tensor_copy(fp_[:], it_p[:])
            nc.vector.tensor_copy(fnr, it_nr[:])
            nc.vector.tensor_copy(fnc, it_nc[:])
            fpj = sp.tile([M, N], F32)
            fpi = sp.tile([M, N], F32)
            nc.gpsimd.memset(fpi[:], 0.0)
            for kk in range(1, PA):
                nc.vector.scalar_tensor_tensor(out=fpi[:], in0=fp_[:], scalar=float(PB * kk),
                                               in1=fpi[:], op0=OP.is_ge, op1=OP.add)
            nc.vector.scalar_tensor_tensor(out=fpj[:], in0=fpi[:], scalar=-float(PB),
                                           in1=fp_[:], op0=OP.mult, op1=OP.add)
            fdi = fnr
            fdj = fnc
            nc.vector.tensor_tensor(out=fdi, in0=fnr, in1=fpi[:], op=OP.subtract)
            nc.vector.tensor_tensor(out=fdj, in0=fnc, in1=fpj[:], op=OP.subtract)
            msk = fpi[:]
            tmp = fpj[:]
            nc.vector.tensor_scalar(out=msk, in0=fdi, scalar1=0.0, scalar2=None, op0=OP.is_ge)
            nc.vector.tensor_scalar(out=tmp, in0=fdi, scalar1=8.0, scalar2=None, op0=OP.is_le)
            nc.vector.tensor_tensor(out=msk, in0=msk, in1=tmp, op=OP.mult)
            nc.vector.tensor_scalar(out=tmp, in0=fdj, scalar1=0.0, scalar2=None, op0=OP.is_ge)
            nc.vector.tensor_tensor(out=msk, in0=msk, in1=tmp, op=OP.mult)
            nc.vector.tensor_scalar(out=tmp, in0=fdj, scalar1=8.0, scalar2=None, op0=OP.is_le)
            nc.vector.tensor_tensor(out=msk, in0=msk, in1=tmp, op=OP.mult)
            nc.vector.tensor_scalar(out=mneg[:], in0=msk, scalar1=1.0, scalar2=-NEG,
                                    op0=OP.subtract, op1=OP.mult)
            for _i in range(4):
                nc.vector.tensor_copy(mneg4[:, _i, :], mneg[:])
            nc.vector.tensor_scalar(out=packm[:], in0=fdi, scalar1=9.0, scalar2=None, op0=OP.mult)
            nc.vector.tensor_tensor(out=packm[:], in0=packm[:], in1=fdj, op=OP.add)
            nc.vector.tensor_tensor(out=packm[:], in0=packm[:], in1=msk, op=OP.mult)

        def emit_front_a(t):
            """DMA + squares for frame t (no PE work)."""
            for k in range(2):
                nc.sync.dma_start(raw[k][:], in_ap[t, k * 128:(k + 1) * 128])
            rawv = [raw[k][:].rearrange("p (r c) -> p r c", r=H) for k in range(2)]
            for cc in range(NCHUNK):
                c0 = cc * CCH
                sqc = sq_t[cc]
                for k in range(2):
                    nc.scalar.activation(
                        sqc[:, k, :].rearrange("p (r c) -> p r c", c=W),
                        rawv[k][:, c0 // W:(c0 + CCH) // W], AF.Square)

        def emit_front_b(t):
            """norm-reduce matmuls + sqrt + recip for frame t."""
            for cc in range(NCHUNK):
                c0 = cc * CCH
                sqc = sq_t[cc]
                nsq = psn.tile([1, CCH], F32, tag="nsq")
                for k in range(2):
                    nc.tensor.matmul(nsq[:], ones[:], sqc[:, k, :],
                                     start=(k == 0), stop=(k == 1))
                nc.scalar.activation(nrow[:, c0:c0 + CCH], nsq[:], AF.Sqrt)
            nc.sync.dma_start(n56[:], nrow[:].rearrange("o (p j) -> o p j", j=56))
            nc.vector.reciprocal(n56[:], n56[:])
            nc.sync.dma_start(nrow[:].rearrange("o (p j) -> o p j", j=56), n56[:])

        def emit_norm(t):
            """Broadcast + normalize planes for frame t (writes xm/xmP[t%2])."""
            par = t % 2
            nc.gpsimd.partition_broadcast(invbc[:], nrow[:], channels=128)
            rawv = [raw[k][:].rearrange("p (r c) -> p r c", r=H) for k in range(2)]
            for b in range(NPJ):
                for k in range(2):
                    xmv = xm[k][par][:].rearrange("p (r c) -> p r c", r=PH)[
                        :, 4:4 + H, 4 + b * PB:4 + (b + 1) * PB]
                    nc.vector.tensor_tensor(
                        out=xmv, in0=rawv[k][:, :, b * PB:(b + 1) * PB],
                        in1=invbc[:].rearrange("p (r c) -> p r c", c=W)[
                            :, :, b * PB:(b + 1) * PB], op=OP.mult)
                    nc.scalar.activation(
                        xmP[k][:, b],
                        xm[k][par][:].rearrange("p (r c) -> p r c", r=PH)[
                            :, 4:4 + H, 4 + b * PB:4 + (b + 1) * PB],
                        AF.Copy)

        def emit_corr_batches(t, blo, bhi):
            """Correlation + argmax for frame t, 4-patch batches [blo, bhi)."""
            cur, prv = t % 2, 1 - (t % 2)
            for B in range(blo, bhi):
                gs = [ps.tile([M, N], F32, tag="gram", name=f"g{t}_{B}_{i}")
                      for i in range(4)]
                masked4 = wp.tile([M, 4, N], F32, tag="masked4", bufs=2)
                gcp4 = wp.tile([M, 4, N], F32, tag="gcp4", bufs=2)
                for i in range(4):
                    P = B * 4 + i
                    Pj, Pi = P // NPI, P % NPI   # band-major iteration
                    g = gs[i]
                    for k in range(2):
                        lhsT = xmP[k][:, Pj, Pi * PA: Pi * PA + PA, :].rearrange(
                            "c a b -> c (a b)")
                        rhs = xm[k][prv][:].rearrange("c (r w) -> c r w", r=PH)[
                            :, Pi * PA: Pi * PA + NR, Pj * PB: Pj * PB + NC_]
                        nc.tensor.matmul(g[:], lhsT, rhs,
                                         start=(k == 0), stop=(k == 1))
                    if B < FUSE_DVE_BATCHES:
                        nc.vector.tensor_tensor(out=masked4[:, i, :], in0=g[:],
                                                in1=mneg[:], op=OP.add)
                    else:
                        nc.scalar.activation(gcp4[:, i, :], g[:], AF.Copy)
                if B >= FUSE_DVE_BATCHES:
                    nc.gpsimd.tensor_tensor(out=masked4[:], in0=gcp4[:],
                                            in1=mneg4[:], op=OP.add)
                nc.vector.tensor_reduce(
                    out=mstore[cur][:, B * 4:(B + 1) * 4], in_=masked4[:],
                    axis=mybir.AxisListType.X, op=OP.max)
                for i in range(4):
                    P = B * 4 + i
                    junk = wp.tile([M, N], F32, tag="junkd", bufs=3)
                    nc.vector.scalar_tensor_tensor(
                        out=junk[:], in0=masked4[:, i, :],
                        scalar=mstore[cur][:, P:P + 1],
                        in1=packm[:], op0=OP.is_ge, op1=OP.mult,
                        accum_out=dstore[cur][:, P:P + 1])
        def emit_decode(t):
            cur = t % 2
            scl = wp.tile([M, NPATCH], F32, tag="scl")
            dx4 = wp.tile([M, NPATCH], F32, tag="dx4")
            dy4 = wp.tile([M, NPATCH], F32, tag="dy4")
            nc.vector.tensor_scalar(out=scl[:], in0=mstore[cur][:], scalar1=0.0, scalar2=None, op0=OP.is_gt)
            nc.vector.tensor_scalar(out=dy4[:], in0=dstore[cur][:], scalar1=9.0, scalar2=None, op0=OP.is_ge)
            for kk in range(2, 9):
                nc.vector.scalar_tensor_tensor(out=dy4[:], in0=dstore[cur][:], scalar=float(9 * kk),
                                               in1=dy4[:], op0=OP.is_ge, op1=OP.add)
            nc.vector.scalar_tensor_tensor(out=dx4[:], in0=dy4[:], scalar=-9.0,
                                           in1=dstore[cur][:], op0=OP.mult, op1=OP.add)
            nc.vector.tensor_scalar(out=dy4[:], in0=dy4[:], scalar1=-4.0, scalar2=None, op0=OP.add)
            nc.vector.tensor_scalar(out=dx4[:], in0=dx4[:], scalar1=-4.0, scalar2=None, op0=OP.add)
            nc.vector.tensor_tensor(out=outbuf[cur][:, 0, :], in0=dy4[:], in1=scl[:], op=OP.mult)
            nc.vector.tensor_tensor(out=outbuf[cur][:, 1, :], in0=dx4[:], in1=scl[:], op=OP.mult)
            nc.vector.tensor_copy(outbuf[cur][:, 2, :], mstore[cur][:])
            nc.sync.dma_start(out_ap[t], outbuf[cur][:])

        # software-pipelined emission with fine-grained interleaving:
        # corr batches of frame t-1 wrap around the front-end of frame t so
        # every engine has early work and the in-order PE queue never blocks.
        emit_front_a(0)
        emit_front_b(0)
        emit_norm(0)
        nc.gpsimd.memset(outbuf[0][:], 0.0)
        nc.sync.dma_start(out_ap[0], outbuf[0][:])
        emit_front_a(1)
        emit_front_b(1)
        emit_norm(1)
        for t in range(2, T):
            emit_corr_batches(t - 1, 0, 1)
            emit_front_a(t)
            emit_corr_batches(t - 1, 1, 3)
            emit_front_b(t)
            emit_corr_batches(t - 1, 3, 7)
            emit_norm(t)
            emit_decode(t - 1)
        emit_corr_batches(T - 1, 0, 7)
        emit_decode(T - 1)

_CACHE = {}


def _get_nc():
    if 'nc' in _CACHE:
        return _CACHE['nc']
    from concourse._compat import axon_active
    nc = bacc.Bacc("TRN2", target_bir_lowering=False, debug=not axon_active(),
                   enable_asserts=False)
    x_d = nc.dram_tensor("x_clip", [T, C, H * W], F32, kind="ExternalInput")
    o_d = nc.dram_tensor("out", [T, M, 3, NPATCH], F32, kind="ExternalOutput")
    with tile.TileContext(nc) as tc:
        _build(tc, o_d.ap(), x_d.ap())
    nc.compile()
    _CACHE['nc'] = nc
    return nc


def _patch_to_image(a):
    """[..., 112, 28] patch layout -> [..., 56, 56]"""
    s = a.shape[:-2]
    a = a.reshape(*s, PA, PB, NPI, NPJ)
    a = np.moveaxis(a, (-4, -3, -2, -1), (-3, -1, -4, -2))
    return np.ascontiguousarray(a.reshape(*s, H, W))


def kernel(x, _trace=False, _trace_kwargs=None):
    x = np.ascontiguousarray(x, dtype=np.float32)
    assert x.shape == (64, 256, 56, 56)
    nc = _get_nc()
    xr = x.reshape(N_CORES, T, C, H * W)
    in_maps = [{"x_clip": np.ascontiguousarray(xr[k])} for k in range(N_CORES)]
    res = run_bass_kernel_spmd(nc, in_maps, core_ids=list(range(N_CORES)),
                               trace=_trace, **(_trace_kwargs or {}))
    # kernel writes patch columns in band-major order (Pj, Pi)
    perm = np.empty(NPATCH, dtype=np.int64)
    for Pj in range(NPJ):
        for Pi in range(NPI):
            perm[Pi * NPJ + Pj] = Pj * NPI + Pi
    outs = []
    for k in range(N_CORES):
        o = res.results[k]["out"]                    # [8, 112, 3, 28]
        o = o.transpose(0, 2, 1, 3)[..., perm]       # [8, 3, 112, 28] row-major P
        outs.append(_patch_to_image(o))              # [8, 3, 56, 56]
    full = np.concatenate(outs, axis=0)              # [64, 3, 56, 56]
    if _trace:
        return full, res
    return full
